# revision 12
# baseline (speedup 1.0000x reference)
"""MultiHeadGAT layer on 8 Trainium2 NeuronCores.

Strategy (graph/data parallel, dst-sharded):
  - Nodes partitioned into 8 ranges (6250/core); each core owns its output
    rows. Edges are routed host-side to the core owning their destination,
    grouped by destination tile (128 dst rows), sorted by source, and split
    into lo/hi halves (src < 25088 or not) so gather indices fit int16.
  - Each core computes a replicated node table in bf16:
        tbl[n] = [xl(n) (128) | s_src(n) (4) | pad]   (256 cols, 512B rows)
    xl = x @ W_lin.T, s_src = xl . att_dst per head (reference applies
    att_dst to source-side features).
  - Edge phase per tile: one dma_gather per half fetches all edge rows
    (per-tile static index counts = max over cores, padded to full 128
    blocks with row-0 dummies); one-hot matrices (from rel data) drive PE
    matmuls for the per-edge s_dst lookup and the scatter-add into PSUM;
    alpha/exp/messages are batched DVE/Scalar ops over the whole tile.
  - Epilogue per tile: softmax divide, +bias, +residual, LayerNorm, ELU.

No collectives: cores fully independent (params + x replicated).
"""

import math

import numpy as np
import ml_dtypes

import concourse.bass as bass
import concourse.bacc as bacc
import concourse.mybir as mybir
from concourse.tile import TileContext
from concourse.bass_utils import run_bass_kernel_spmd

F32 = mybir.dt.float32
BF16 = mybir.dt.bfloat16
I16 = mybir.dt.int16
AF = mybir.ActivationFunctionType
OP = mybir.AluOpType
AX = mybir.AxisListType

H, C = 4, 32
HC = H * C          # 128
IN_CH = 128
ED = 16
NEG_SLOPE = 0.2
LN_EPS = 1e-5
P = 128
HALF = 25088        # nodes per table half (196 tiles of 128)
EW = 256            # table row width (bf16 elems); gathered payload 512B
SW = 132            # significant row cols: xl(128) | s_src(4)
B1M, B2M = 18, 18   # max index blocks per half

FULL_CFG = dict(n_nodes=50000, n_cores=8, n_edges=1600000)


def derive_cfg(cfg):
    n, cores = cfg["n_nodes"], cfg["n_cores"]
    npc = n // cores
    assert npc * cores == n
    tiles = math.ceil(npc / P)
    npad = tiles * P
    nt_tbl = 2 * HALF // P
    assert nt_tbl * P >= n
    return dict(cfg, npc=npc, tiles=tiles, npad=npad, nt_tbl=nt_tbl)


# --------------------------------------------------------------------------
# host-side edge routing (index bookkeeping + layout only)
# --------------------------------------------------------------------------

def host_prep(x, edge_index, edge_attr, W_lin, W_edge, att_src, att_dst,
              att_edge, bias, ln_gamma, ln_beta, cfg):
    cfg = derive_cfg(cfg)
    cores, npc, tiles = cfg["n_cores"], cfg["npc"], cfg["tiles"]
    n = cfg["n_nodes"]
    MM = B1M + B2M
    cap = MM * P

    src = np.asarray(edge_index[0], np.int64)
    dst = np.asarray(edge_index[1], np.int64)
    ea = np.asarray(edge_attr, np.float32)

    core_of = dst // npc
    local = dst - core_of * npc
    tile_of = local // P
    rel = local - tile_of * P
    is_hi = (src >= HALF).astype(np.int64)

    # per-(core,tile,half) counts -> per-tile static block counts
    grp = (core_of * tiles + tile_of) * 2 + is_hi
    counts = np.bincount(grp, minlength=cores * tiles * 2).reshape(
        cores, tiles, 2)
    B1s = np.ceil(counts[:, :, 0].max(axis=0) / P).astype(int)
    B2s = np.ceil(counts[:, :, 1].max(axis=0) / P).astype(int)
    assert (B1s <= B1M).all() and (B2s <= B2M).all()

    # sort by (core, tile, half, src) -> slot position within each group
    key = grp * (2 ** 26) + src
    order = np.argsort(key, kind="stable")
    g_core = core_of[order]
    g_tile = tile_of[order]
    g_hi = is_hi[order]
    g_src = src[order]
    g_rel = rel[order]
    g_ea = ea[order]

    grp_s = (g_core * tiles + g_tile) * 2 + g_hi
    cflat = counts.reshape(-1)
    starts = np.zeros_like(cflat)
    np.cumsum(cflat[:-1], out=starts[1:])
    within = np.arange(len(g_src)) - starts[grp_s]
    # hi edges start at chunk B1s[tile]
    k_slot = within + g_hi * (B1s[g_tile] * P)

    idx_flat = np.zeros((cores, tiles, cap), np.int16)     # dummy -> row 0
    rel_flat = np.full((cores, tiles, cap), -1.0, ml_dtypes.bfloat16)
    ea_flat = np.zeros((cores, tiles, cap, ED), ml_dtypes.bfloat16)
    idx_flat[g_core, g_tile, k_slot] = (g_src - g_hi * HALF).astype(np.int16)
    rel_flat[g_core, g_tile, k_slot] = g_rel.astype(np.float32)
    ea_flat[g_core, g_tile, k_slot] = g_ea.astype(ml_dtypes.bfloat16)

    # idx16 wrap per tile: lo slots [0, B1_t*128) at cols [0, B1_t*8); hi
    # slots [B1_t*128, M_t*128) at cols [B1M*8, B1M*8 + B2_t*8).
    # flat k -> partition k%16 (replicated x8), col k//16
    idx16 = np.zeros((cores, tiles, 128, MM * 8), np.int16)
    for t in range(tiles):
        b1 = int(B1s[t])
        m_t = b1 + int(B2s[t])
        kv = np.arange(m_t * P)
        in_lo = kv < b1 * P
        kk = np.where(in_lo, kv, kv - b1 * P)
        col = np.where(in_lo, kk // 16, B1M * 8 + kk // 16)
        for r in range(8):
            idx16[:, t, r * 16 + kk % 16, col] = idx_flat[:, t, :m_t * P]

    relp = np.ascontiguousarray(
        rel_flat.reshape(cores, tiles, MM, P).transpose(0, 1, 3, 2))
    relT = rel_flat.reshape(cores, tiles, 1, cap)
    ea_sw = np.ascontiguousarray(
        ea_flat.reshape(cores, tiles, MM, P, ED).transpose(0, 1, 3, 2, 4)
        .reshape(cores, tiles, P, MM * ED))

    # packed per-tile edge stream: [idx16 | relp | ea] int16 units
    edat = np.concatenate([
        idx16,
        relp.view(np.int16),
        ea_sw.view(np.int16),
    ], axis=3)

    x = np.asarray(x, np.float32)
    x_pad = np.zeros((2 * HALF, IN_CH), np.float32)
    x_pad[:n] = x
    npad = cfg["npad"]
    xres = np.zeros((cores, npad, IN_CH), np.float32)
    for c in range(cores):
        xres[c, :npc] = x[c * npc:(c + 1) * npc]
    xgT = np.ascontiguousarray(x_pad.T.astype(ml_dtypes.bfloat16))
    xresT16 = np.ascontiguousarray(
        xres.transpose(0, 2, 1).astype(ml_dtypes.bfloat16))

    iota_r = np.arange(P, dtype=np.float32).astype(
        ml_dtypes.bfloat16).reshape(1, P)
    iota_c = np.ascontiguousarray(iota_r.reshape(P, 1))

    in_maps = []
    for c in range(cores):
        in_maps.append(dict(
            xgT=xgT,
            xres=np.ascontiguousarray(xres[c]),
            xresT=xresT16[c],
            edat=np.ascontiguousarray(edat[c]),
            relT=np.ascontiguousarray(relT[c]),
            W_lin=np.asarray(W_lin, np.float32),
            W_linT=np.ascontiguousarray(np.asarray(W_lin, np.float32).T),
            W_edge=np.asarray(W_edge, np.float32),
            a_src=np.asarray(att_src, np.float32).reshape(HC, 1),
            a_dst=np.asarray(att_dst, np.float32).reshape(HC, 1),
            a_edge=np.asarray(att_edge, np.float32).reshape(HC, 1),
            bias=np.asarray(bias, np.float32).reshape(1, HC),
            ln_gamma=np.asarray(ln_gamma, np.float32).reshape(1, HC),
            ln_beta=np.asarray(ln_beta, np.float32).reshape(1, HC),
            iota_r=iota_r,
            iota_c=iota_c,
        ))
    return in_maps, cfg, (B1s, B2s)


# --------------------------------------------------------------------------
# device program
# --------------------------------------------------------------------------

def build_program(cfg, tile_blocks, num_devices=None):
    cfg = derive_cfg(cfg)
    tiles, npad, nt_tbl = cfg["tiles"], cfg["npad"], cfg["nt_tbl"]
    B1s, B2s = tile_blocks
    MM = B1M + B2M
    cap = MM * P
    EDW = MM * 8 + MM + MM * ED    # packed edat width (int16 units)
    TB = 4                         # phase-B tiles per batch

    nc = bacc.Bacc("TRN2", target_bir_lowering=False, debug=False,
                   num_devices=num_devices or cfg["n_cores"])

    dp = nc.declare_dram_parameter
    xgT_d = dp("xgT", [IN_CH, 2 * HALF], BF16, isOutput=False)
    xres_d = dp("xres", [npad, IN_CH], F32, isOutput=False)
    xresT_d = dp("xresT", [IN_CH, npad], BF16, isOutput=False)
    edat_d = dp("edat", [tiles, 128, EDW], I16, isOutput=False)
    relT_d = dp("relT", [tiles, 1, cap], BF16, isOutput=False)
    wl_d = dp("W_lin", [HC, IN_CH], F32, isOutput=False)
    wlT_d = dp("W_linT", [IN_CH, HC], F32, isOutput=False)
    we_d = dp("W_edge", [HC, ED], F32, isOutput=False)
    asrc_d = dp("a_src", [HC, 1], F32, isOutput=False)
    adst_d = dp("a_dst", [HC, 1], F32, isOutput=False)
    aedge_d = dp("a_edge", [HC, 1], F32, isOutput=False)
    bias_d = dp("bias", [1, HC], F32, isOutput=False)
    gamma_d = dp("ln_gamma", [1, HC], F32, isOutput=False)
    beta_d = dp("ln_beta", [1, HC], F32, isOutput=False)
    iotar_d = dp("iota_r", [1, P], BF16, isOutput=False)
    iotac_d = dp("iota_c", [P, 1], BF16, isOutput=False)
    out_d = dp("out", [npad, HC], F32, isOutput=True)

    tbl_lo = nc.dram_tensor("tbl_lo", [HALF, EW], BF16)
    tbl_hi = nc.dram_tensor("tbl_hi", [HALF, EW], BF16)
    ct_dram = nc.dram_tensor("ct_scratch", [H, ED], BF16)

    with TileContext(nc) as tc:
        with (
            tc.tile_pool(name="const", bufs=1) as cpool,
            tc.tile_pool(name="pb", bufs=3) as bpool,
            tc.tile_pool(name="psB", bufs=3, space="PSUM") as psB,
            tc.tile_pool(name="ld", bufs=2) as lpool,
            tc.tile_pool(name="gath", bufs=3) as gpool,
            tc.tile_pool(name="wk", bufs=2) as wpool,
            tc.tile_pool(name="psS", bufs=2, space="PSUM") as psS,
            tc.tile_pool(name="psA", bufs=2, space="PSUM") as psA,
        ):
            # ================= phase A: constants =========================
            iotab = cpool.tile([P, P], BF16, tag="iotab")
            nc.sync.dma_start(out=iotab[:], in_=iotar_d[:].to_broadcast([P, P]))
            iota_c = cpool.tile([P, 1], BF16, tag="iota_c")
            nc.sync.dma_start(out=iota_c[:], in_=iotac_d[:])

            wl_sb = cpool.tile([HC, IN_CH], F32, tag="wl")
            nc.sync.dma_start(out=wl_sb[:], in_=wl_d[:])
            wl16 = cpool.tile([HC, IN_CH], BF16, tag="wl16")
            nc.vector.tensor_copy(out=wl16[:], in_=wl_sb[:])
            wlT_sb = cpool.tile([IN_CH, HC], F32, tag="wlT")
            nc.sync.dma_start(out=wlT_sb[:], in_=wlT_d[:])
            we_sb = cpool.tile([HC, ED], F32, tag="we")
            nc.sync.dma_start(out=we_sb[:], in_=we_d[:])
            we16 = cpool.tile([HC, ED], BF16, tag="we16")
            nc.vector.tensor_copy(out=we16[:], in_=we_sb[:])

            att = {}
            for name, d in (("asrc", asrc_d), ("adst", adst_d),
                            ("aedge", aedge_d)):
                a = cpool.tile([HC, 1], F32, tag=name)
                nc.sync.dma_start(out=a[:], in_=d[:])
                a8 = cpool.tile([HC, H], BF16, tag=name + "8")
                nc.gpsimd.memset(a8[:], 0.0)
                for h in range(H):
                    sl = slice(h * C, (h + 1) * C)
                    nc.vector.tensor_copy(out=a8[sl, h:h + 1], in_=a[sl, :])
                att[name] = a8

            # rhsBT = [W_lin^T | v_dst] (bf16, [128, 132])
            rhsBT = cpool.tile([IN_CH, SW], BF16, tag="rhsbt")
            nc.vector.tensor_copy(out=rhsBT[:, 0:HC], in_=wlT_sb[:])
            b4_ps = psB.tile([IN_CH, H], F32, tag="ps")
            nc.tensor.matmul(out=b4_ps[:], lhsT=wl16[:], rhs=att["adst"][:],
                             start=True, stop=True)
            nc.scalar.copy(out=rhsBT[:, HC:SW], in_=b4_ps[:])
            a4s_ps = psB.tile([IN_CH, H], F32, tag="ps")
            nc.tensor.matmul(out=a4s_ps[:], lhsT=wl16[:], rhs=att["asrc"][:],
                             start=True, stop=True)
            a4s = cpool.tile([IN_CH, H], BF16, tag="a4s")
            nc.vector.tensor_copy(out=a4s[:], in_=a4s_ps[:])

            c_ps = psB.tile([ED, H], F32, tag="ps")
            nc.tensor.matmul(out=c_ps[:], lhsT=we16[:], rhs=att["aedge"][:],
                             start=True, stop=True)
            c16 = cpool.tile([ED, H], BF16, tag="c16")
            nc.vector.tensor_copy(out=c16[:], in_=c_ps[:])
            nc.sync.dma_start(out=ct_dram[:].rearrange("h d -> d h"),
                              in_=c16[:])
            ctb = cpool.tile([P, H * ED], BF16, tag="ctb")
            nc.sync.dma_start(
                out=ctb[:],
                in_=ct_dram[:].rearrange("h d -> (h d)").unsqueeze(0)
                              .to_broadcast([P, H * ED]))

            bias_b = cpool.tile([P, HC], F32, tag="bias_b")
            nc.sync.dma_start(out=bias_b[:], in_=bias_d[:].to_broadcast([P, HC]))
            gamma_b = cpool.tile([P, HC], F32, tag="gamma_b")
            nc.sync.dma_start(out=gamma_b[:],
                              in_=gamma_d[:].to_broadcast([P, HC]))
            beta_b = cpool.tile([P, HC], F32, tag="beta_b")
            nc.sync.dma_start(out=beta_b[:], in_=beta_d[:].to_broadcast([P, HC]))
            eps_t = cpool.tile([P, 1], F32, tag="eps_t")
            nc.gpsimd.memset(eps_t[:], LN_EPS)
            tiny_t = cpool.tile([P, 1], F32, tag="tiny_t")
            nc.gpsimd.memset(tiny_t[:], 1e-16)
            zeros_b = cpool.tile([P, HC], F32, tag="zeros_b")
            nc.gpsimd.memset(zeros_b[:], 0.0)

            s_own = cpool.tile([P, tiles * H], BF16, tag="s_own")

            # ================= phase B: node table ========================
            for tb in range(nt_tbl // TB):
                r0 = tb * TB * P
                xT4 = bpool.tile([P, TB * P], BF16, tag="xT4")
                nc.scalar.dma_start(out=xT4[:], in_=xgT_d[:, r0:r0 + TB * P])
                rows4 = bpool.tile([P, TB * SW], BF16, tag="rows4")
                for a in range(TB):
                    row_ps = psB.tile([P, SW], F32, tag="ps")
                    nc.tensor.matmul(out=row_ps[:],
                                     lhsT=xT4[:, a * P:(a + 1) * P],
                                     rhs=rhsBT[:], start=True, stop=True)
                    nc.vector.tensor_copy(out=rows4[:, a * SW:(a + 1) * SW],
                                          in_=row_ps[:])
                t0 = tb * TB
                tdst = tbl_lo if t0 < nt_tbl // 2 else tbl_hi
                toff = (t0 % (nt_tbl // 2)) * P
                nc.sync.dma_start(
                    out=tdst[toff:toff + TB * P, 0:SW].rearrange(
                        "(a p) w -> p a w", p=P),
                    in_=rows4[:].rearrange("p (a w) -> p a w", a=TB))

            # ============ phase B2: s_own (dst-side scores) ===============
            for t in range(tiles):
                xrT = bpool.tile([P, P], BF16, tag="xrT")
                nc.scalar.dma_start(out=xrT[:],
                                    in_=xresT_d[:, t * P:(t + 1) * P])
                so_ps = psB.tile([P, H], F32, tag="ps")
                nc.tensor.matmul(out=so_ps[:], lhsT=xrT[:], rhs=a4s[:],
                                 start=True, stop=True)
                nc.vector.tensor_copy(out=s_own[:, t * H:(t + 1) * H],
                                      in_=so_ps[:])

            # gathers must not start before the table is fully written
            tc.strict_bb_all_engine_barrier()

            # ================= phase C: edges =============================
            for t in range(tiles):
                B1, B2 = int(B1s[t]), int(B2s[t])
                M = B1 + B2
                ed = lpool.tile([128, EDW], I16, tag="ed")
                nc.sync.dma_start(out=ed[:], in_=edat_d[t])
                rp = ed[:, MM * 8:MM * 8 + M].bitcast(BF16)
                eat = ed[:, MM * 9:MM * 9 + M * ED].bitcast(BF16)
                rb = lpool.tile([P, cap], BF16, tag="rb")
                nc.scalar.dma_start(
                    out=rb[:, 0:M * P],
                    in_=relT_d[t, :, 0:M * P].to_broadcast([P, M * P]))

                g = gpool.tile([P, MM * EW], BF16, tag="g")
                gv = g[:].rearrange("p (m w) -> p m w", w=EW)
                nc.gpsimd.dma_gather(gv[:, 0:B1, :], tbl_lo[:],
                                     ed[:, 0:B1 * 8], B1 * P, B1 * P, EW,
                                     single_packet=False)
                nc.gpsimd.dma_gather(gv[:, B1:M, :], tbl_hi[:],
                                     ed[:, B1M * 8:B1M * 8 + B2 * 8],
                                     B2 * P, B2 * P, EW,
                                     single_packet=False)

                oh = wpool.tile([P, cap], BF16, tag="oh")
                nc.vector.tensor_tensor(
                    out=oh[:, 0:M * P].rearrange("p (m q) -> p m q", q=P),
                    in0=rp.unsqueeze(2).to_broadcast([P, M, P]),
                    in1=iotab[:].unsqueeze(1).to_broadcast([P, M, P]),
                    op=OP.is_equal)
                ohdt = wpool.tile([P, cap], BF16, tag="ohdt")
                nc.vector.tensor_tensor(
                    out=ohdt[:, 0:M * P].rearrange("p (m q) -> p m q", q=P),
                    in0=iota_c[:].unsqueeze(1).to_broadcast([P, M, P]),
                    in1=rb[:, 0:M * P].rearrange("p (m q) -> p m q", q=P),
                    op=OP.is_equal)

                sdst_ps = psS.tile([P, MM * H], F32, tag="sdst")
                for m in range(M):
                    nc.tensor.matmul(
                        out=sdst_ps[:, m * H:(m + 1) * H],
                        lhsT=ohdt[:, m * P:(m + 1) * P],
                        rhs=s_own[:, t * H:(t + 1) * H],
                        start=True, stop=True)

                # eterm: one 4D mult + one reduce (h-major), then assemble
                prod = wpool.tile([P, H * MM * ED], BF16, tag="prod")
                pv = prod[:].rearrange("p (h m e) -> p h m e", h=H, e=ED)
                ev = eat.rearrange("p (m e) -> p m e", e=ED)
                nc.vector.tensor_tensor(
                    out=pv[:, :, 0:M, :],
                    in0=ev.unsqueeze(1).to_broadcast([P, H, M, ED]),
                    in1=ctb[:].rearrange("p (h e) -> p h e", e=ED)
                        .unsqueeze(2).to_broadcast([P, H, M, ED]),
                    op=OP.mult)
                et_hm = wpool.tile([P, H * MM], F32, tag="et_hm")
                nc.vector.reduce_sum(
                    out=et_hm[:].rearrange("p (h m) -> p h m", h=H)
                        [:, :, 0:M].unsqueeze(3),
                    in_=pv[:, :, 0:M, :], axis=AX.X)

                alf = wpool.tile([P, MM * H], F32, tag="alf")
                av = alf[:].rearrange("p (m h) -> p m h", h=H)
                nc.vector.tensor_tensor(
                    out=av[:, 0:M, :],
                    in0=et_hm[:].rearrange("p (h m) -> p m h", h=H)[:, 0:M, :],
                    in1=gv[:, 0:M, HC:HC + H], op=OP.add)
                nc.vector.tensor_add(out=alf[:, 0:M * H],
                                     in0=alf[:, 0:M * H],
                                     in1=sdst_ps[:, 0:M * H])
                nc.vector.scalar_tensor_tensor(
                    out=alf[:, 0:M * H], in0=alf[:, 0:M * H],
                    scalar=NEG_SLOPE, in1=alf[:, 0:M * H],
                    op0=OP.mult, op1=OP.max)
                ex16 = wpool.tile([P, MM * H], BF16, tag="ex16")
                nc.scalar.activation(out=ex16[:, 0:M * H],
                                     in_=alf[:, 0:M * H], func=AF.Exp)

                # messages [P, M, 132]: 4D mult + denom copy
                msg = wpool.tile([P, MM * SW], BF16, tag="msg")
                mv = msg[:].rearrange("p (m w) -> p m w", w=SW)
                m4 = mv[:, 0:M, 0:HC].rearrange("p m (h c) -> p m h c", c=C)
                xv = ex16[:].rearrange("p (m h) -> p m h", h=H)
                nc.vector.tensor_tensor(
                    out=m4[:, :, :, :],
                    in0=gv[:, 0:M, 0:HC].rearrange(
                        "p m (h c) -> p m h c", c=C),
                    in1=xv[:, 0:M, :].unsqueeze(3).to_broadcast([P, M, H, C]),
                    op=OP.mult)
                nc.scalar.copy(out=mv[:, 0:M, HC:HC + H], in_=xv[:, 0:M, :])

                acc = psA.tile([P, SW], F32, tag="acc")
                for m in range(M):
                    nc.tensor.matmul(out=acc[:],
                                     lhsT=oh[:, m * P:(m + 1) * P],
                                     rhs=msg[:, m * SW:(m + 1) * SW],
                                     start=(m == 0), stop=(m == M - 1))

                # ---------------- epilogue ------------------------------
                den = wpool.tile([P, H], F32, tag="den")
                nc.scalar.activation(out=den[:], in_=acc[:, HC:HC + H],
                                     func=AF.Identity, bias=tiny_t[:, 0:1])
                rden = wpool.tile([P, H], F32, tag="rden")
                nc.vector.reciprocal(out=rden[:], in_=den[:])
                o = wpool.tile([P, HC], F32, tag="o")
                nc.vector.tensor_tensor(
                    out=o[:].rearrange("p (h c) -> p h c", c=C),
                    in0=acc[:, 0:HC].rearrange("p (h c) -> p h c", c=C),
                    in1=rden[:].unsqueeze(2).to_broadcast([P, H, C]),
                    op=OP.mult)
                nc.vector.tensor_add(out=o[:], in0=o[:], in1=bias_b[:])
                xr = wpool.tile([P, HC], F32, tag="xr")
                nc.sync.dma_start(out=xr[:], in_=xres_d[t * P:(t + 1) * P, :])
                nc.vector.tensor_add(out=o[:], in0=o[:], in1=xr[:])

                mu = wpool.tile([P, 1], F32, tag="mu")
                nc.vector.reduce_sum(out=mu[:], in_=o[:], axis=AX.X)
                negmu = wpool.tile([P, 1], F32, tag="negmu")
                nc.scalar.mul(out=negmu[:], in_=mu[:], mul=-1.0 / HC)
                ctr = wpool.tile([P, HC], F32, tag="ctr")
                nc.scalar.activation(out=ctr[:], in_=o[:], func=AF.Identity,
                                     bias=negmu[:, 0:1])
                sq = wpool.tile([P, HC], F32, tag="sq")
                var = wpool.tile([P, 1], F32, tag="var")
                nc.vector.tensor_mul(out=sq[:], in0=ctr[:], in1=ctr[:])
                nc.vector.reduce_sum(out=var[:], in_=sq[:], axis=AX.X)
                std = wpool.tile([P, 1], F32, tag="std")
                nc.scalar.activation(out=std[:], in_=var[:], func=AF.Sqrt,
                                     scale=1.0 / HC, bias=eps_t[:, 0:1])
                rstd = wpool.tile([P, 1], F32, tag="rstd")
                nc.vector.reciprocal(out=rstd[:], in_=std[:])
                nrm = wpool.tile([P, HC], F32, tag="nrm")
                nc.scalar.activation(out=nrm[:], in_=ctr[:], func=AF.Copy,
                                     scale=rstd[:, 0:1])
                nc.vector.tensor_mul(out=nrm[:], in0=nrm[:], in1=gamma_b[:])
                nc.vector.tensor_add(out=nrm[:], in0=nrm[:], in1=beta_b[:])

                ex2 = wpool.tile([P, HC], F32, tag="ex2")
                nc.scalar.activation(out=ex2[:], in_=nrm[:], func=AF.Exp)
                nc.vector.scalar_tensor_tensor(
                    out=ex2[:], in0=ex2[:], scalar=-1.0, in1=zeros_b[:],
                    op0=OP.add, op1=OP.min)
                rl = wpool.tile([P, HC], F32, tag="rl")
                nc.scalar.activation(out=rl[:], in_=nrm[:], func=AF.Relu)
                nc.vector.tensor_add(out=rl[:], in0=rl[:], in1=ex2[:])
                nc.sync.dma_start(out=out_d[t * P:(t + 1) * P, :], in_=rl[:])

    nc.compile()
    return nc


# --------------------------------------------------------------------------
# entry point
# --------------------------------------------------------------------------

def kernel(**inputs) -> np.ndarray:
    cfg = FULL_CFG
    in_maps, dcfg, tile_blocks = host_prep(cfg=cfg, **inputs)
    nc = build_program(cfg, tile_blocks)
    cores = cfg["n_cores"]
    res = run_bass_kernel_spmd(nc, in_maps, list(range(cores)))
    npc = dcfg["npc"]
    parts = [res.results[c]["out"][:npc] for c in range(cores)]
    return np.concatenate(parts, axis=0).astype(np.float32)
